# revision 36
# baseline (speedup 1.0000x reference)
"""CBFGraphNet Trainium2 kernel.

Math notes (exact rewrites of the reference, no approximation beyond fp
reassociation):

  The reference returns a scalar computed from nodes[0] only ("drone").
  Edge states are never updated from node states, so the final value
  depends only on:
    - node_feats[0]
    - S0 = sum of edge_feats rows whose receiver == 0
    - c0 = number of edges whose receiver == 0
    - the (tiny) weight matrices
  via segment_sum linearity:
    segment_sum(edge_feats @ W + b)[0] == S0 @ W + c0 * b

Device work (8 NeuronCores, edges sharded evenly, SPMD):

  Primary path ("compaction"): the host casts receivers to uint16
  (receiver==0 iff low16==0, except the false-positive value 65536 which
  the host filters), halving DMA bytes.  Each core streams its
  [128 x 3125] uint16 slice window by window on a SINGLE dynamic DMA
  queue (every declared queue expands to 16 physical queues whose
  semaphores the NEFF postamble must reset one by one - extra queues
  cost ~1us of postamble each), and the vector engine chases the stream
  with find_index8 over each window.  The host turns (window, slot) hits
  into global edge ids, re-checks them against the true int32 receivers
  (dropping 65536 aliases), gathers those few edge_feats rows, and
  finishes the O(1) MLP.

  Fallback path ("streaming", used only if some window saturates all 8
  find8 slots so the index list could be incomplete): stream all
  edge_feats too and compute S0 as a masked sum on-device.
"""

import sys

if "/opt/trn_rl_repo" not in sys.path:
    sys.path.insert(0, "/opt/trn_rl_repo")

import numpy as np

N_NODES = 100_000
N_EDGES = 3_200_000
F_IN = 16
HID = 64
N_CORES = 8
P = 128

EC = N_EDGES // N_CORES          # 400_000 edges per core
JPC = EC // P                    # 3125 edges per partition
M = 625                          # streaming path: edges/partition/chunk
NCHUNK = JPC // M                # 5

_CACHE: dict = {}
LAST_RESULTS = None              # BassKernelResults from the latest run

# Input windows, in DVE processing order: (start, end, queue) with
# queue in {"sync", "scalar"} - the engine whose HWDGE ring streams the
# window.  Windows sharing a queue stream FIFO on that ring; the two
# rings race each other off the shared DMA engines.  A small first
# window starts the DVE early; sizes are chosen so each window's
# DMA-completion semaphore fires at or before the DVE's (~1.24 ns/col)
# arrival at that window.
WINDOWS = [
    (0, 192, "sync"),
    (192, 704, "scalar"),
    (704, 1600, "scalar"),
    (1600, 2624, "sync"),
    (2624, 3125, "sync"),
]
NW = len(WINDOWS)
RING_ORDER = {"sync": [0, 3, 4], "scalar": [1, 2]}
OC = 8 * NW          # ixb cols: one 8-slot index group per window
# The NEFF postamble (engine barriers + per-physical-queue sem resets)
# runs for several us after the last kernel instruction, giving the
# final index write-out ample time to land without an explicit
# completion wait; measured correct and deterministic across runs.
FINAL_WAIT = False


def _build_compact():
    """Raw-Block (no TileContext) receivers scan: per window, top-8
    match positions of value 0 via find_index8.  One DMA ring, FIFO;
    the vector engine chases the stream window by window."""
    import concourse.bacc as bacc
    import concourse.mybir as mybir

    u16 = mybir.dt.uint16
    u32 = mybir.dt.uint32

    nc = bacc.Bacc("TRN2", target_bir_lowering=False,
                   enable_partition_id=False)
    rvs = [nc.declare_dram_parameter(f"rv{h}", [P, b - a], u16,
                                      isOutput=False)
           for h, (a, b, q) in enumerate(WINDOWS)]
    oidx = nc.declare_dram_parameter("oidx", [P, OC], u32, isOutput=True)
    with (
        nc.sbuf_tensor([P, JPC], u16) as rt,
        nc.sbuf_tensor([P, 8], u16) as zeros8,
        nc.sbuf_tensor([P, OC], u32) as ixb,
        nc.semaphore("in0") as in0,
        nc.semaphore("in1") as in1,
        nc.semaphore("in2") as in2,
        nc.semaphore("in3") as in3,
        nc.semaphore("in4") as in4,
        nc.semaphore("vec_done") as vec_done,
        nc.semaphore("vchain") as vchain,
        nc.semaphore("dma_out") as dma_out,
        nc.Block(no_gpsimd_drain=True) as block,
    ):
        ins = [in0, in1, in2, in3, in4]
        assert NW <= 5

        def emit_in_dmas(eng, qname):
            for h in RING_ORDER[qname]:
                a, b, q = WINDOWS[h]
                assert q == qname
                eng.dma_start(out=rt[:, a:b], in_=rvs[h][:]).then_inc(
                    ins[h], 16)

        @block.sync
        def _(sync):
            emit_in_dmas(sync, "sync")
            # windows 0-2 indices go out under the shadow of the tail
            # find8s; the final 16 cols wait for the last find8.
            sync.wait_ge(vec_done, 3)
            sync.dma_start(out=oidx[:, 0:24], in_=ixb[:, 0:24]
                           ).then_inc(dma_out, 16)
            sync.wait_ge(vec_done, NW)
            sync.dma_start(out=oidx[:, 24:OC], in_=ixb[:, 24:OC]
                           ).then_inc(dma_out, 16)
            if FINAL_WAIT:
                sync.wait_ge(dma_out, 32)

        @block.scalar
        def _(scalar):
            emit_in_dmas(scalar, "scalar")

        @block.vector
        def _(vector):
            vector.memset(zeros8[:], 0).then_inc(vchain, 1)
            for h, (a, b, q) in enumerate(WINDOWS):
                vector.wait_ge(ins[h], 16)
                if h == 0:
                    vector.wait_ge(vchain, 1)
                vector.max_index(
                    ixb[:, 8 * h:8 * h + 8], zeros8[:],
                    rt[:, a:b]).then_inc(vec_done, 1)
    nc.compile()
    return nc


def _build_stream():
    import concourse.bacc as bacc
    import concourse.mybir as mybir
    from concourse.tile import TileContext

    f32 = mybir.dt.float32
    i32 = mybir.dt.int32

    nc = bacc.Bacc("TRN2", target_bir_lowering=False)
    ef = nc.declare_dram_parameter("ef", [P, JPC * F_IN], f32, isOutput=False)
    rv = nc.declare_dram_parameter("rv", [P, JPC], i32, isOutput=False)
    out = nc.declare_dram_parameter("out", [P, F_IN + 1], f32, isOutput=True)

    with TileContext(nc) as tc:
        with tc.tile_pool(name="x", bufs=2) as xp, \
             tc.tile_pool(name="small", bufs=2) as sp, \
             tc.tile_pool(name="persist", bufs=1) as pp:
            acc = pp.tile([P, F_IN + 1], f32)
            nc.vector.memset(acc[:], 0.0)
            for c in range(NCHUNK):
                x = xp.tile([P, M * F_IN], f32)
                r = sp.tile([P, M], i32, tag="recv")
                mk = sp.tile([P, M], f32, tag="mask")
                red = sp.tile([P, F_IN + 1], f32, tag="red")
                nc.sync.dma_start(
                    out=x[:], in_=ef[:, c * M * F_IN:(c + 1) * M * F_IN])
                nc.sync.dma_start(out=r[:], in_=rv[:, c * M:(c + 1) * M])
                nc.vector.tensor_scalar(
                    out=mk[:], in0=r[:], scalar1=0, scalar2=None,
                    op0=mybir.AluOpType.is_equal)
                x3 = x[:].rearrange("p (j f) -> p j f", f=F_IN)
                nc.vector.tensor_tensor(
                    out=x3, in0=x3, in1=mk[:].broadcast_to((P, M, F_IN)),
                    op=mybir.AluOpType.mult)
                nc.vector.tensor_reduce(
                    out=red[:, 0:F_IN],
                    in_=x[:].rearrange("p (j f) -> p f j", f=F_IN),
                    axis=mybir.AxisListType.X, op=mybir.AluOpType.add)
                nc.vector.tensor_reduce(
                    out=red[:, F_IN:F_IN + 1], in_=mk[:],
                    axis=mybir.AxisListType.X, op=mybir.AluOpType.add)
                nc.vector.tensor_tensor(
                    out=acc[:], in0=acc[:], in1=red[:],
                    op=mybir.AluOpType.add)
            nc.sync.dma_start(out=out[:], in_=acc[:])
    nc.compile()
    return nc


def _get(name, builder):
    if name not in _CACHE:
        _CACHE[name] = builder()
    return _CACHE[name]


def _finish(S0, c0, node_feats, node_W, node_b, edge_W, edge_b,
            msg_W0, msg_b0, msg_W1, msg_b1,
            upd_W0, upd_b0, upd_W1, upd_b1,
            cbf_W1, cbf_b1, cbf_W2, cbf_b2):
    # O(1) finish: node-0 slice of the reference network.
    e_enc = S0 @ edge_W + c0 * edge_b
    n0 = node_feats[0] @ node_W + node_b
    for mW, mb, uW, ub in ((msg_W0, msg_b0, upd_W0, upd_b0),
                           (msg_W1, msg_b1, upd_W1, upd_b1)):
        agg = e_enc @ mW + c0 * mb
        n0 = np.maximum((n0 + agg) @ uW + ub, np.float32(0.0))
    h = np.maximum(n0 @ cbf_W1 + cbf_b1, np.float32(0.0))
    val = h @ cbf_W2 + cbf_b2
    return np.float32(val[0])


def kernel(node_feats, edge_feats, receivers,
           node_W, node_b, edge_W, edge_b,
           msg_W0, msg_b0, msg_W1, msg_b1,
           upd_W0, upd_b0, upd_W1, upd_b1,
           cbf_W1, cbf_b1, cbf_W2, cbf_b2,
           _trace=False, _trace_cores=None, _force_stream=False):
    global LAST_RESULTS
    from concourse.bass_utils import run_bass_kernel_spmd

    node_feats = np.asarray(node_feats, dtype=np.float32)
    node_W, node_b = np.asarray(node_W), np.asarray(node_b)
    edge_W, edge_b = np.asarray(edge_W), np.asarray(edge_b)
    msg_W0, msg_b0 = np.asarray(msg_W0), np.asarray(msg_b0)
    msg_W1, msg_b1 = np.asarray(msg_W1), np.asarray(msg_b1)
    upd_W0, upd_b0 = np.asarray(upd_W0), np.asarray(upd_b0)
    upd_W1, upd_b1 = np.asarray(upd_W1), np.asarray(upd_b1)
    cbf_W1, cbf_b1 = np.asarray(cbf_W1), np.asarray(cbf_b1)
    cbf_W2, cbf_b2 = np.asarray(cbf_W2), np.asarray(cbf_b2)
    edge_feats = np.ascontiguousarray(edge_feats, dtype=np.float32)
    receivers = np.ascontiguousarray(receivers, dtype=np.int32)
    rv_sh = receivers.reshape(N_CORES, P, JPC)
    # uint16 view for the device scan: receiver==0 iff low16==0 except
    # for the false-positive value 65536, verified away host-side below.
    rv16_sh = receivers.astype(np.uint16).reshape(N_CORES, P, JPC)

    weights = dict(
        node_feats=node_feats, node_W=node_W, node_b=node_b,
        edge_W=edge_W, edge_b=edge_b,
        msg_W0=msg_W0, msg_b0=msg_b0, msg_W1=msg_W1, msg_b1=msg_b1,
        upd_W0=upd_W0, upd_b0=upd_b0, upd_W1=upd_W1, upd_b1=upd_b1,
        cbf_W1=cbf_W1, cbf_b1=cbf_b1, cbf_W2=cbf_W2, cbf_b2=cbf_b2)

    if not _force_stream:
        nc = _get("compact", _build_compact)
        in_maps = [
            {f"rv{h}": np.ascontiguousarray(rv16_sh[k][:, a:b])
             for h, (a, b, q) in enumerate(WINDOWS)}
            for k in range(N_CORES)]
        res = run_bass_kernel_spmd(
            nc, in_maps, list(range(N_CORES)),
            trace=_trace, trace_cores=_trace_cores)
        LAST_RESULTS = res
        raw = [np.asarray(r["oidx"]).reshape(P, OC) for r in res.results]
        idxs = np.stack(raw).reshape(N_CORES, P, NW, 8).astype(np.uint32)
        # find_index8 writes -1 (0xFFFFFFFF) for unmatched query slots;
        # matched slots are trailing-free, so the count is the # of valid.
        counts = (idxs != np.uint32(0xFFFFFFFF)).sum(axis=3)        # [8,P,NW]
        if counts.max() < 8:
            # 8 hits in one window-row would mean a possibly-truncated
            # index list, so only trust strictly-below-saturation rows.
            cand = []
            ks, ps, hs = np.nonzero(counts)
            for k, p, h in zip(ks, ps, hs):
                c = counts[k, p, h]
                js = idxs[k, p, h, :c].astype(np.int64) + WINDOWS[h][0]
                cand.append((k * P + p) * JPC + js)
            if cand:
                e = np.concatenate(cand)
                e = e[receivers[e] == 0]    # drop uint16 aliases (65536)
            else:
                e = np.empty(0, np.int64)
            S0 = edge_feats[e].sum(axis=0, dtype=np.float32)
            c0 = np.float32(len(e))
            return _finish(S0, c0, **weights)
        # else: saturated window-row - index list may be incomplete,
        # fall through to the streaming path.

    nc = _get("stream", _build_stream)
    ef_sh = edge_feats.reshape(N_CORES, P, JPC * F_IN)
    in_maps = [{"ef": ef_sh[k], "rv": rv_sh[k]} for k in range(N_CORES)]
    res = run_bass_kernel_spmd(
        nc, in_maps, list(range(N_CORES)),
        trace=_trace, trace_cores=_trace_cores)
    LAST_RESULTS = res
    partials = np.stack([np.asarray(r["out"]) for r in res.results])
    partials = partials.sum(axis=(0, 1), dtype=np.float64)
    S0 = partials[:F_IN].astype(np.float32)
    c0 = np.float32(partials[F_IN])
    return _finish(S0, c0, **weights)


# revision 37
# speedup vs baseline: 1.1022x; 1.1022x over previous
"""CBFGraphNet Trainium2 kernel.

Math notes (exact rewrites of the reference, no approximation beyond fp
reassociation):

  The reference returns a scalar computed from nodes[0] only ("drone").
  Edge states are never updated from node states, so the final value
  depends only on:
    - node_feats[0]
    - S0 = sum of edge_feats rows whose receiver == 0
    - c0 = number of edges whose receiver == 0
    - the (tiny) weight matrices
  via segment_sum linearity:
    segment_sum(edge_feats @ W + b)[0] == S0 @ W + c0 * b

Device work (8 NeuronCores, edges sharded evenly, SPMD):

  Primary path ("compaction"): the host casts receivers to uint16
  (receiver==0 iff low16==0, except the false-positive value 65536 which
  the host filters), halving DMA bytes.  Each core streams its
  [128 x 3125] uint16 slice window by window on a SINGLE dynamic DMA
  queue (every declared queue expands to 16 physical queues whose
  semaphores the NEFF postamble must reset one by one - extra queues
  cost ~1us of postamble each), and the vector engine chases the stream
  with find_index8 over each window.  The host turns (window, slot) hits
  into global edge ids, re-checks them against the true int32 receivers
  (dropping 65536 aliases), gathers those few edge_feats rows, and
  finishes the O(1) MLP.

  Fallback path ("streaming", used only if some window saturates all 8
  find8 slots so the index list could be incomplete): stream all
  edge_feats too and compute S0 as a masked sum on-device.
"""

import sys

if "/opt/trn_rl_repo" not in sys.path:
    sys.path.insert(0, "/opt/trn_rl_repo")

import numpy as np

N_NODES = 100_000
N_EDGES = 3_200_000
F_IN = 16
HID = 64
N_CORES = 8
P = 128

EC = N_EDGES // N_CORES          # 400_000 edges per core
JPC = EC // P                    # 3125 edges per partition
M = 625                          # streaming path: edges/partition/chunk
NCHUNK = JPC // M                # 5

_CACHE: dict = {}
LAST_RESULTS = None              # BassKernelResults from the latest run

# Input windows, in DVE processing order: (start, end, queue) with
# queue in {"sync", "scalar"} - the engine whose HWDGE ring streams the
# window.  Windows sharing a queue stream FIFO on that ring; the two
# rings race each other off the shared DMA engines.  A small first
# window starts the DVE early; sizes are chosen so each window's
# DMA-completion semaphore fires at or before the DVE's (~1.24 ns/col)
# arrival at that window.
WINDOWS = [
    (0, 192, "sync"),
    (192, 704, "scalar"),
    (704, 1600, "sync"),
    (1600, 2624, "scalar"),
    (2624, 3125, "sync"),
]
NW = len(WINDOWS)
RING_ORDER = {"sync": [0, 2, 4], "scalar": [1, 3]}
OC = 8 * NW          # ixb cols: one 8-slot index group per window
# The NEFF postamble (engine barriers + per-physical-queue sem resets)
# runs for several us after the last kernel instruction, giving the
# final index write-out ample time to land without an explicit
# completion wait; measured correct and deterministic across runs.
FINAL_WAIT = False


def _build_compact():
    """Raw-Block (no TileContext) receivers scan: per window, top-8
    match positions of value 0 via find_index8.  One DMA ring, FIFO;
    the vector engine chases the stream window by window."""
    import concourse.bacc as bacc
    import concourse.mybir as mybir

    u16 = mybir.dt.uint16
    u32 = mybir.dt.uint32

    nc = bacc.Bacc("TRN2", target_bir_lowering=False,
                   enable_partition_id=False)
    rvs = [nc.declare_dram_parameter(f"rv{h}", [P, b - a], u16,
                                      isOutput=False)
           for h, (a, b, q) in enumerate(WINDOWS)]
    oidx = nc.declare_dram_parameter("oidx", [P, OC], u32, isOutput=True)
    with (
        nc.sbuf_tensor([P, JPC], u16) as rt,
        nc.sbuf_tensor([P, 8], u16) as zeros8,
        nc.sbuf_tensor([P, OC], u32) as ixb,
        nc.semaphore("in0") as in0,
        nc.semaphore("in1") as in1,
        nc.semaphore("in2") as in2,
        nc.semaphore("in3") as in3,
        nc.semaphore("in4") as in4,
        nc.semaphore("vec_done") as vec_done,
        nc.semaphore("vchain") as vchain,
        nc.semaphore("dma_out") as dma_out,
        nc.Block(no_gpsimd_drain=True) as block,
    ):
        ins = [in0, in1, in2, in3, in4]
        assert NW <= 5

        def emit_in_dmas(eng, qname):
            for h in RING_ORDER[qname]:
                a, b, q = WINDOWS[h]
                assert q == qname
                eng.dma_start(out=rt[:, a:b], in_=rvs[h][:]).then_inc(
                    ins[h], 16)

        @block.sync
        def _(sync):
            emit_in_dmas(sync, "sync")
            # windows 0-2 indices go out under the shadow of the tail
            # find8s; the final 16 cols wait for the last find8.
            sync.wait_ge(vec_done, 3)
            sync.dma_start(out=oidx[:, 0:24], in_=ixb[:, 0:24]
                           ).then_inc(dma_out, 16)
            sync.wait_ge(vec_done, NW)
            sync.dma_start(out=oidx[:, 24:OC], in_=ixb[:, 24:OC]
                           ).then_inc(dma_out, 16)
            if FINAL_WAIT:
                sync.wait_ge(dma_out, 32)

        @block.scalar
        def _(scalar):
            emit_in_dmas(scalar, "scalar")

        @block.vector
        def _(vector):
            vector.memset(zeros8[:], 0).then_inc(vchain, 1)
            for h, (a, b, q) in enumerate(WINDOWS):
                vector.wait_ge(ins[h], 16)
                if h == 0:
                    vector.wait_ge(vchain, 1)
                vector.max_index(
                    ixb[:, 8 * h:8 * h + 8], zeros8[:],
                    rt[:, a:b]).then_inc(vec_done, 1)
    nc.compile()
    return nc


def _build_stream():
    import concourse.bacc as bacc
    import concourse.mybir as mybir
    from concourse.tile import TileContext

    f32 = mybir.dt.float32
    i32 = mybir.dt.int32

    nc = bacc.Bacc("TRN2", target_bir_lowering=False)
    ef = nc.declare_dram_parameter("ef", [P, JPC * F_IN], f32, isOutput=False)
    rv = nc.declare_dram_parameter("rv", [P, JPC], i32, isOutput=False)
    out = nc.declare_dram_parameter("out", [P, F_IN + 1], f32, isOutput=True)

    with TileContext(nc) as tc:
        with tc.tile_pool(name="x", bufs=2) as xp, \
             tc.tile_pool(name="small", bufs=2) as sp, \
             tc.tile_pool(name="persist", bufs=1) as pp:
            acc = pp.tile([P, F_IN + 1], f32)
            nc.vector.memset(acc[:], 0.0)
            for c in range(NCHUNK):
                x = xp.tile([P, M * F_IN], f32)
                r = sp.tile([P, M], i32, tag="recv")
                mk = sp.tile([P, M], f32, tag="mask")
                red = sp.tile([P, F_IN + 1], f32, tag="red")
                nc.sync.dma_start(
                    out=x[:], in_=ef[:, c * M * F_IN:(c + 1) * M * F_IN])
                nc.sync.dma_start(out=r[:], in_=rv[:, c * M:(c + 1) * M])
                nc.vector.tensor_scalar(
                    out=mk[:], in0=r[:], scalar1=0, scalar2=None,
                    op0=mybir.AluOpType.is_equal)
                x3 = x[:].rearrange("p (j f) -> p j f", f=F_IN)
                nc.vector.tensor_tensor(
                    out=x3, in0=x3, in1=mk[:].broadcast_to((P, M, F_IN)),
                    op=mybir.AluOpType.mult)
                nc.vector.tensor_reduce(
                    out=red[:, 0:F_IN],
                    in_=x[:].rearrange("p (j f) -> p f j", f=F_IN),
                    axis=mybir.AxisListType.X, op=mybir.AluOpType.add)
                nc.vector.tensor_reduce(
                    out=red[:, F_IN:F_IN + 1], in_=mk[:],
                    axis=mybir.AxisListType.X, op=mybir.AluOpType.add)
                nc.vector.tensor_tensor(
                    out=acc[:], in0=acc[:], in1=red[:],
                    op=mybir.AluOpType.add)
            nc.sync.dma_start(out=out[:], in_=acc[:])
    nc.compile()
    return nc


def _get(name, builder):
    if name not in _CACHE:
        _CACHE[name] = builder()
    return _CACHE[name]


def _finish(S0, c0, node_feats, node_W, node_b, edge_W, edge_b,
            msg_W0, msg_b0, msg_W1, msg_b1,
            upd_W0, upd_b0, upd_W1, upd_b1,
            cbf_W1, cbf_b1, cbf_W2, cbf_b2):
    # O(1) finish: node-0 slice of the reference network.
    e_enc = S0 @ edge_W + c0 * edge_b
    n0 = node_feats[0] @ node_W + node_b
    for mW, mb, uW, ub in ((msg_W0, msg_b0, upd_W0, upd_b0),
                           (msg_W1, msg_b1, upd_W1, upd_b1)):
        agg = e_enc @ mW + c0 * mb
        n0 = np.maximum((n0 + agg) @ uW + ub, np.float32(0.0))
    h = np.maximum(n0 @ cbf_W1 + cbf_b1, np.float32(0.0))
    val = h @ cbf_W2 + cbf_b2
    return np.float32(val[0])


def kernel(node_feats, edge_feats, receivers,
           node_W, node_b, edge_W, edge_b,
           msg_W0, msg_b0, msg_W1, msg_b1,
           upd_W0, upd_b0, upd_W1, upd_b1,
           cbf_W1, cbf_b1, cbf_W2, cbf_b2,
           _trace=False, _trace_cores=None, _force_stream=False):
    global LAST_RESULTS
    from concourse.bass_utils import run_bass_kernel_spmd

    node_feats = np.asarray(node_feats, dtype=np.float32)
    node_W, node_b = np.asarray(node_W), np.asarray(node_b)
    edge_W, edge_b = np.asarray(edge_W), np.asarray(edge_b)
    msg_W0, msg_b0 = np.asarray(msg_W0), np.asarray(msg_b0)
    msg_W1, msg_b1 = np.asarray(msg_W1), np.asarray(msg_b1)
    upd_W0, upd_b0 = np.asarray(upd_W0), np.asarray(upd_b0)
    upd_W1, upd_b1 = np.asarray(upd_W1), np.asarray(upd_b1)
    cbf_W1, cbf_b1 = np.asarray(cbf_W1), np.asarray(cbf_b1)
    cbf_W2, cbf_b2 = np.asarray(cbf_W2), np.asarray(cbf_b2)
    edge_feats = np.ascontiguousarray(edge_feats, dtype=np.float32)
    receivers = np.ascontiguousarray(receivers, dtype=np.int32)
    rv_sh = receivers.reshape(N_CORES, P, JPC)
    # uint16 view for the device scan: receiver==0 iff low16==0 except
    # for the false-positive value 65536, verified away host-side below.
    rv16_sh = receivers.astype(np.uint16).reshape(N_CORES, P, JPC)

    weights = dict(
        node_feats=node_feats, node_W=node_W, node_b=node_b,
        edge_W=edge_W, edge_b=edge_b,
        msg_W0=msg_W0, msg_b0=msg_b0, msg_W1=msg_W1, msg_b1=msg_b1,
        upd_W0=upd_W0, upd_b0=upd_b0, upd_W1=upd_W1, upd_b1=upd_b1,
        cbf_W1=cbf_W1, cbf_b1=cbf_b1, cbf_W2=cbf_W2, cbf_b2=cbf_b2)

    if not _force_stream:
        nc = _get("compact", _build_compact)
        in_maps = [
            {f"rv{h}": np.ascontiguousarray(rv16_sh[k][:, a:b])
             for h, (a, b, q) in enumerate(WINDOWS)}
            for k in range(N_CORES)]
        res = run_bass_kernel_spmd(
            nc, in_maps, list(range(N_CORES)),
            trace=_trace, trace_cores=_trace_cores)
        LAST_RESULTS = res
        raw = [np.asarray(r["oidx"]).reshape(P, OC) for r in res.results]
        idxs = np.stack(raw).reshape(N_CORES, P, NW, 8).astype(np.uint32)
        # find_index8 writes -1 (0xFFFFFFFF) for unmatched query slots;
        # matched slots are trailing-free, so the count is the # of valid.
        counts = (idxs != np.uint32(0xFFFFFFFF)).sum(axis=3)        # [8,P,NW]
        if counts.max() < 8:
            # 8 hits in one window-row would mean a possibly-truncated
            # index list, so only trust strictly-below-saturation rows.
            cand = []
            ks, ps, hs = np.nonzero(counts)
            for k, p, h in zip(ks, ps, hs):
                c = counts[k, p, h]
                js = idxs[k, p, h, :c].astype(np.int64) + WINDOWS[h][0]
                cand.append((k * P + p) * JPC + js)
            if cand:
                e = np.concatenate(cand)
                e = e[receivers[e] == 0]    # drop uint16 aliases (65536)
            else:
                e = np.empty(0, np.int64)
            S0 = edge_feats[e].sum(axis=0, dtype=np.float32)
            c0 = np.float32(len(e))
            return _finish(S0, c0, **weights)
        # else: saturated window-row - index list may be incomplete,
        # fall through to the streaming path.

    nc = _get("stream", _build_stream)
    ef_sh = edge_feats.reshape(N_CORES, P, JPC * F_IN)
    in_maps = [{"ef": ef_sh[k], "rv": rv_sh[k]} for k in range(N_CORES)]
    res = run_bass_kernel_spmd(
        nc, in_maps, list(range(N_CORES)),
        trace=_trace, trace_cores=_trace_cores)
    LAST_RESULTS = res
    partials = np.stack([np.asarray(r["out"]) for r in res.results])
    partials = partials.sum(axis=(0, 1), dtype=np.float64)
    S0 = partials[:F_IN].astype(np.float32)
    c0 = np.float32(partials[F_IN])
    return _finish(S0, c0, **weights)
